# revision 11
# baseline (speedup 1.0000x reference)
"""Trainium2 Bass kernel for CLSControlledDynamicBlock.

Computation (per reference):
  x = cls_token[:, 0, :]                      # (16, 768)
  h = relu(x @ W1 + b1)                       # (16, 192)
  params = tanh(h @ W2 + b2)                  # (16, 36864)
  w = params.reshape(16, 64, 64, 3, 3)        # per-sample conv kernels
  out[s] = conv2d_same(features[s], w[s]) + features[s]

Two SPMD launches on 8 NeuronCores:
  Phase A: the params MLP, sharded over the 36864 output columns
           (each core: full x/W1 fp32, a 192x4608 slice of W2 in bf16).
  Host:    reorder params (2.3MB) into per-sample weight tiles
           wT[s, kx, ky, ci, co].
  Phase B: data-parallel conv, 2 samples per core. SBUF partitions are
           (sample, ci): sample A on partitions 0-63 / PE quadrant
           (0,0), sample B on partitions 64-127 / quadrant (64,64),
           running concurrently on the PE array. Work is pipelined in 4
           row bands: one 128-partition feature DMA per band (sync
           ring), f32->bf16 cast into a zero-padded image (DVE/GPSIMD
           alternating), 7 PSUM chunks of 4 output rows x 9 taps, fp32
           residual add into an output band buffer, one out-DMA per
           band on the scalar (ACT) ring.
"""

import numpy as np
import ml_dtypes

import concourse.bass as bass
import concourse.mybir as mybir
import concourse.tile as tile
from concourse.tile_rust import add_dep_helper
from concourse import bacc
from concourse.bass_utils import run_bass_kernel_spmd

F32 = mybir.dt.float32
BF16 = mybir.dt.bfloat16
AF = mybir.ActivationFunctionType

B, EMB, CIN, COUT, K, H, W = 16, 768, 64, 64, 3, 112, 112
HID = EMB // 4  # 192
TOTAL = COUT * CIN * K * K  # 36864
NCORES = 8
SH = TOTAL // NCORES  # 4608 params columns per core
MT = SH // 128  # 36 output tiles of 128 rows
KO = EMB // 128  # 6 contraction tiles for x @ W1

HP = H + 2  # 114 padded width/height
NB = 4  # row bands
BAND_OUT = H // NB  # 28 output rows per band
CH = 4  # output rows per PSUM chunk
CPB = BAND_OUT // CH  # 7 chunks per band


def build_phase_a():
    nc = bacc.Bacc("TRN2", target_bir_lowering=False, debug=False,
                   num_devices=NCORES)
    # Pre-swizzled on host: xp[p, ko*B+n] = x[n, ko*128+p], similarly W1.
    xp = nc.dram_tensor("xp", [128, KO * B], F32, kind="ExternalInput")
    w1p = nc.dram_tensor("w1p", [128, KO * HID], F32, kind="ExternalInput")
    b1 = nc.dram_tensor("b1", [HID, 1], F32, kind="ExternalInput")
    W2s = nc.dram_tensor("W2s", [HID, SH], BF16, kind="ExternalInput")
    b2p = nc.dram_tensor("b2p", [128, MT], F32, kind="ExternalInput")
    # pout[p, mt*B+n] = paramsT[mt*128+p, n]
    pout = nc.dram_tensor("pout", [128, MT * B], F32, kind="ExternalOutput")

    NW2C = 3  # W2 column chunks (DMA/compute overlap)
    CW = SH // NW2C  # 1536 columns per chunk
    MT_PER_C = CW // 128  # 12

    with tile.TileContext(nc) as tc:
        with (
            tc.tile_pool(name="const", bufs=1) as const,
            tc.tile_pool(name="psum", bufs=1, space="PSUM") as psum,
        ):
            # Small inputs first so they don't queue behind the W2 slabs.
            xT_sb = const.tile([128, KO, B], F32, tag="xT")
            nc.sync.dma_start(xT_sb.rearrange("p ko n -> p (ko n)"), xp.ap())
            W1_sb = const.tile([128, KO, HID], F32, tag="W1")
            nc.sync.dma_start(W1_sb.rearrange("p ko m -> p (ko m)"), w1p.ap())
            b1a = const.tile([128, 1], F32, tag="b1a")
            nc.sync.dma_start(b1a[:], b1.ap()[0:128, :])
            b1b = const.tile([64, 1], F32, tag="b1b")
            nc.sync.dma_start(b1b[:], b1.ap()[128:HID, :])
            b2_sb = const.tile([128, MT], F32, tag="b2")
            nc.sync.dma_start(b2_sb[:], b2p.ap())

            # W2 slice (bf16), chunked; rows 0-127 and 128-191 separately.
            w2a = []
            w2b = []
            for c in range(NW2C):
                ta = const.tile([128, CW], BF16, tag=f"w2a{c}")
                nc.sync.dma_start(ta[:], W2s.ap()[0:128, c * CW:(c + 1) * CW])
                w2a.append(ta)
                tb = const.tile([64, CW], BF16, tag=f"w2b{c}")
                nc.sync.dma_start(tb[:], W2s.ap()[128:HID, c * CW:(c + 1) * CW])
                w2b.append(tb)

            # hT = relu(W1.T @ x.T + b1), shape (192, 16) as 128 + 64 rows.
            ph1 = psum.tile([128, B], F32, tag="ph")
            for k in range(KO):
                nc.tensor.matmul(ph1[:], W1_sb[:, k, 0:128], xT_sb[:, k, :],
                                 start=(k == 0), stop=(k == KO - 1))
            ph2 = psum.tile([64, B], F32, tag="ph2")
            for k in range(KO):
                nc.tensor.matmul(ph2[:], W1_sb[:, k, 128:HID], xT_sb[:, k, :],
                                 start=(k == 0), stop=(k == KO - 1))
            hT1 = const.tile([128, B], F32, tag="hT1")
            nc.scalar.activation(hT1[:], ph1[:], AF.Relu, bias=b1a[:])
            hT2 = const.tile([64, B], F32, tag="hT2")
            nc.scalar.activation(hT2[:], ph2[:], AF.Relu, bias=b1b[:])
            hb1 = const.tile([128, B], BF16, tag="hb1")
            nc.vector.tensor_copy(out=hb1[:], in_=hT1[:])
            hb2 = const.tile([64, B], BF16, tag="hb2")
            nc.vector.tensor_copy(out=hb2[:], in_=hT2[:])

            # paramsT tile mt = tanh(W2s[:, mt].T @ hT + b2s[mt])
            outp = const.tile([128, MT, B], F32, tag="outp")
            for mt in range(MT):
                c, i = divmod(mt, MT_PER_C)
                pp = psum.tile([128, B], F32, tag="pp", bufs=6)
                nc.tensor.matmul(pp[:], w2a[c][:, i * 128:(i + 1) * 128],
                                 hb1[:], start=True, stop=False)
                nc.tensor.matmul(pp[:], w2b[c][:, i * 128:(i + 1) * 128],
                                 hb2[:], start=False, stop=True)
                nc.scalar.activation(outp[:, mt, :], pp[:], AF.Tanh,
                                     bias=b2_sb[:, mt:mt + 1])
            nc.sync.dma_start(pout.ap(), outp.rearrange("p mt n -> p (mt n)"))

    nc.compile()
    return nc


def build_phase_b():
    nc = bacc.Bacc("TRN2", target_bir_lowering=False, debug=False,
                   num_devices=NCORES)
    feat = nc.dram_tensor("feat", [2, CIN, H, W], F32, kind="ExternalInput")
    # Pair weights wp[p, s, kx, co]: for sample A (s=0) partitions are
    # (ky=0 ci | ky=1 ci); for sample B (s=1) they are (ky=1 | ky=0) --
    # matching the flipped plane layout below. ws[p, kx, co] holds the
    # ky=2 taps: partitions (A ci | B ci).
    wp = nc.dram_tensor("wp", [128, 2, K, COUT], BF16, kind="ExternalInput")
    ws = nc.dram_tensor("ws", [128, K, COUT], BF16, kind="ExternalInput")
    out = nc.dram_tensor("out", [2, COUT, H, W], F32, kind="ExternalOutput")
    outp = out.ap().rearrange("s c r x -> (s c) r x")

    # Output-row bands: a small first band fills the pipeline quickly.
    BANDS = [(0, 8), (8, 24), (32, 28), (60, 28), (88, 24)]
    NBD = len(BANDS)

    with tile.TileContext(nc) as tc:
        with (
            tc.tile_pool(name="const", bufs=1) as const,
            tc.tile_pool(name="bands", bufs=1) as bands,
            tc.tile_pool(name="outs", bufs=2) as outs,
            tc.tile_pool(name="psum", bufs=1, space="PSUM") as psum,
        ):
            wpair = const.tile([128, 2, K, COUT], BF16, tag="wpair")
            nc.sync.dma_start(wpair[:], wp.ap())
            wsing = const.tile([128, K, COUT], BF16, tag="wsing")
            nc.sync.dma_start(wsing[:], ws.ap())

            # PE warm-up: ~3.5us of junk matmuls so HAM is at full clock
            # when the first real matmul issues.
            junk = const.tile([128, 128], BF16, tag="junk")
            nc.vector.memset(junk[:], 0.0)
            jps = psum.tile([128, 128], F32, tag="jps")
            for i in range(36):
                nc.tensor.matmul(jps[:], junk[:], junk[:],
                                 start=(i == 0), stop=(i == 35),
                                 skip_group_check=True)

            # Padded bf16 feature planes, filled straight from HBM by
            # SWDGE cast-DMAs (fp32 -> bf16 during the transfer; the
            # single SWDGE context also serializes bands for free).
            # Plane band b covers padded rows [s0, s0+n+3), i.e. feature
            # rows [s0-1, s0+n+2) clipped; local row lp = padded - s0.
            # planeA: partitions 0-63 = F (sample A), 64-127 = G = F one
            # row up. planeB flipped: 0-63 = G (sample B), 64-127 = F.
            plA, plB = [], []
            for b, (s0, n) in enumerate(BANDS):
                plA.append(bands.tile([128, n + 3, HP], BF16, tag=f"plA{b}",
                                      name=f"plA{b}"))
                plB.append(bands.tile([128, n + 3, HP], BF16, tag=f"plB{b}",
                                      name=f"plB{b}"))

            for b, (s0, n) in enumerate(BANDS):
                r0 = max(0, s0 - 1)
                r1 = min(H, s0 + n + 2)
                PR = n + 3
                lp0 = 1 if b == 0 else 0
                nr = r1 - r0
                fa = plA[b][0:64]     # F plane, sample A
                fb = plB[b][64:128]   # F plane, sample B
                # zero borders of the F planes (G inherits zeros via shift)
                for f in (fa, fb):
                    nc.vector.memset(f[:, :, 0:1], 0.0)
                    nc.vector.memset(f[:, :, HP - 1:HP], 0.0)
                    if b == 0:
                        nc.vector.memset(f[:, 0:1, :], 0.0)
                    if b == NBD - 1:
                        nc.vector.memset(f[:, lp0 + nr:PR, :], 0.0)
                # feature load + cast in one SWDGE DMA per sample
                nc.gpsimd.dma_start(fa[:, lp0:lp0 + nr, 1:1 + W],
                                    feat.ap()[0, :, r0:r1, :])
                nc.gpsimd.dma_start(fb[:, lp0:lp0 + nr, 1:1 + W],
                                    feat.ap()[1, :, r0:r1, :])
                # G = F shifted one row up (SBUF->SBUF on the idle SP ring)
                nc.sync.dma_start(plA[b][64:128, 0:PR - 1, :],
                                  plA[b][0:64, 1:PR, :])
                nc.sync.dma_start(plB[b][0:64, 0:PR - 1, :],
                                  plB[b][64:128, 1:PR, :])

            for b, (s0, n) in enumerate(BANDS):
                cpb = n // CH
                ob = outs.tile([128, n, W], F32, tag=f"ob{b % 2}",
                               name=f"ob{b}")
                pss = [psum.tile([128, CH, W], F32, tag="ps", bufs=7,
                                 name=f"ps{b}_{j}") for j in range(cpb)]
                for t in range(2 * K):  # 3 pair slots then 3 single slots
                    kx = t % K
                    for j in range(cpb):
                        for s in range(2):
                            sl = slice(s * 64, (s + 1) * 64)
                            pl = (plA, plB)[s][b]
                            if t < K:  # ky={0,1} pair, K=128
                                lhsT = wpair[:, s, kx, :]
                                rhs = pl[:, CH * j:CH * j + CH, kx:kx + W]
                            else:  # ky=2 single, K=64 on the F plane
                                lhsT = wsing[sl, kx, :]
                                rhs = pl[sl, CH * j + 2:CH * j + 2 + CH,
                                         kx:kx + W]
                            nc.tensor.matmul(
                                pss[j][sl], lhsT, rhs,
                                start=(t == 0), stop=(t == 2 * K - 1),
                                tile_position=(0 if t < K else s * 64,
                                               s * 64),
                                skip_group_check=True)
                for j in range(cpb):
                    # residual adds from the bf16 F planes (per sample)
                    lj = CH * j
                    nc.vector.tensor_add(
                        out=ob[0:64, lj:lj + CH, :], in0=pss[j][0:64],
                        in1=plA[b][0:64, lj + 1:lj + 1 + CH, 1:1 + W])
                    nc.vector.tensor_add(
                        out=ob[64:128, lj:lj + CH, :], in0=pss[j][64:128],
                        in1=plB[b][64:128, lj + 1:lj + 1 + CH, 1:1 + W])
                    if b == NBD - 1 and j % 2 == 1:
                        # stream the last band out in quarters to cut the
                        # kernel tail
                        y0 = s0 + lj
                        nc.scalar.dma_start(
                            outp[:, y0 - CH:y0 + CH, :],
                            ob[:, lj - CH:lj + CH, :])
                if b != NBD - 1:
                    nc.scalar.dma_start(outp[:, s0:s0 + n, :], ob[:])

    nc.compile()
    return nc


def prep_a_inputs(cls_token, W1, b1, W2, b2):
    x = cls_token[:, 0, :]  # (16, 768)
    xp = np.ascontiguousarray(
        x.T.reshape(KO, 128, B).transpose(1, 0, 2).reshape(128, KO * B))
    w1p = np.ascontiguousarray(
        W1.reshape(KO, 128, HID).transpose(1, 0, 2).reshape(128, KO * HID))
    b1c = np.ascontiguousarray(b1.reshape(HID, 1))
    W2b = W2.astype(ml_dtypes.bfloat16)
    in_a = []
    for j in range(NCORES):
        sl = slice(j * SH, (j + 1) * SH)
        in_a.append({
            "xp": xp,
            "w1p": w1p,
            "b1": b1c,
            "W2s": np.ascontiguousarray(W2b[:, sl]),
            "b2p": np.ascontiguousarray(b2[sl].reshape(MT, 128).T),
        })
    return in_a


def params_from_a(res_a):
    # pout[p, mt*B+n] = paramsT[mt*128+p, n] -> (TOTAL, B)
    slabs = []
    for j in range(NCORES):
        po = res_a.results[j]["pout"].reshape(128, MT, B)
        slabs.append(po.transpose(1, 0, 2).reshape(SH, B))
    return np.concatenate(slabs, axis=0)


def wT_from_params(paramsT):
    # rows are (co, ci, ky, kx). Build per-core pair/single weight slabs:
    #   T[s, ky, ci, kx, co] = w[s][co, ci, ky, kx]
    T = np.ascontiguousarray(
        paramsT.reshape(COUT, CIN, K, K, B).transpose(4, 2, 1, 3, 0)
    ).astype(ml_dtypes.bfloat16)
    wps, wss = [], []
    for j in range(NCORES):
        A, Bm = T[2 * j], T[2 * j + 1]
        wpc = np.empty((128, 2, K, COUT), dtype=ml_dtypes.bfloat16)
        wpc[:64, 0] = A[0]; wpc[64:, 0] = A[1]   # A: (F=ky0 | G=ky1)
        wpc[:64, 1] = Bm[1]; wpc[64:, 1] = Bm[0]  # B flipped: (G=ky1 | F=ky0)
        wsc = np.empty((128, K, COUT), dtype=ml_dtypes.bfloat16)
        wsc[:64] = A[2]; wsc[64:] = Bm[2]
        wps.append(np.ascontiguousarray(wpc))
        wss.append(np.ascontiguousarray(wsc))
    return wps, wss


def prep_b_inputs(features, wT):
    wps, wss = wT
    return [
        {"feat": features[2 * j:2 * j + 2], "wp": wps[j], "ws": wss[j]}
        for j in range(NCORES)
    ]


_cache = {}


def _get(name, builder):
    if name not in _cache:
        _cache[name] = builder()
    return _cache[name]


def kernel(cls_token, features, W1, b1, W2, b2):
    cls_token = np.asarray(cls_token, dtype=np.float32)
    features = np.ascontiguousarray(np.asarray(features, dtype=np.float32))
    W1 = np.ascontiguousarray(np.asarray(W1, dtype=np.float32))
    b1 = np.asarray(b1, dtype=np.float32)
    W2 = np.asarray(W2, dtype=np.float32)
    b2 = np.asarray(b2, dtype=np.float32)

    ncA = _get("A", build_phase_a)
    ncB = _get("B", build_phase_b)
    cores = list(range(NCORES))

    in_a = prep_a_inputs(cls_token, W1, b1, W2, b2)
    res_a = run_bass_kernel_spmd(ncA, in_a, core_ids=cores)
    paramsT = params_from_a(res_a)
    wT = wT_from_params(paramsT)

    in_b = prep_b_inputs(features, wT)
    res_b = run_bass_kernel_spmd(ncB, in_b, core_ids=cores)
    out = np.concatenate(
        [res_b.results[j]["out"] for j in range(NCORES)], axis=0)
    return out


# revision 13
# speedup vs baseline: 1.1459x; 1.1459x over previous
"""Trainium2 Bass kernel for CLSControlledDynamicBlock.

Computation (per reference):
  x = cls_token[:, 0, :]                      # (16, 768)
  h = relu(x @ W1 + b1)                       # (16, 192)
  params = tanh(h @ W2 + b2)                  # (16, 36864)
  w = params.reshape(16, 64, 64, 3, 3)        # per-sample conv kernels
  out[s] = conv2d_same(features[s], w[s]) + features[s]

Two SPMD launches on 8 NeuronCores:
  Phase A: the params MLP, sharded over the 36864 output columns
           (each core: full x/W1 fp32, a 192x4608 slice of W2 in bf16).
  Host:    reorder params (2.3MB) into per-sample weight tiles
           wT[s, kx, ky, ci, co].
  Phase B: data-parallel conv, 2 samples per core. SBUF partitions are
           (sample, ci): sample A on partitions 0-63 / PE quadrant
           (0,0), sample B on partitions 64-127 / quadrant (64,64),
           running concurrently on the PE array. Work is pipelined in 4
           row bands: one 128-partition feature DMA per band (sync
           ring), f32->bf16 cast into a zero-padded image (DVE/GPSIMD
           alternating), 7 PSUM chunks of 4 output rows x 9 taps, fp32
           residual add into an output band buffer, one out-DMA per
           band on the scalar (ACT) ring.
"""

import numpy as np
import ml_dtypes

import concourse.bass as bass
import concourse.mybir as mybir
import concourse.tile as tile
from concourse.tile_rust import add_dep_helper
from concourse import bacc
from concourse.bass_utils import run_bass_kernel_spmd

F32 = mybir.dt.float32
BF16 = mybir.dt.bfloat16
AF = mybir.ActivationFunctionType

B, EMB, CIN, COUT, K, H, W = 16, 768, 64, 64, 3, 112, 112
HID = EMB // 4  # 192
TOTAL = COUT * CIN * K * K  # 36864
NCORES = 8
SH = TOTAL // NCORES  # 4608 params columns per core
MT = SH // 128  # 36 output tiles of 128 rows
KO = EMB // 128  # 6 contraction tiles for x @ W1

HP = H + 2  # 114 padded width/height
NB = 4  # row bands
BAND_OUT = H // NB  # 28 output rows per band
CH = 4  # output rows per PSUM chunk
CPB = BAND_OUT // CH  # 7 chunks per band


def build_phase_a():
    nc = bacc.Bacc("TRN2", target_bir_lowering=False, debug=False,
                   num_devices=NCORES)
    # Pre-swizzled on host: xp[p, ko*B+n] = x[n, ko*128+p], similarly W1.
    xp = nc.dram_tensor("xp", [128, KO * B], F32, kind="ExternalInput")
    w1p = nc.dram_tensor("w1p", [128, KO * HID], F32, kind="ExternalInput")
    b1 = nc.dram_tensor("b1", [HID, 1], F32, kind="ExternalInput")
    W2s = nc.dram_tensor("W2s", [HID, SH], BF16, kind="ExternalInput")
    b2p = nc.dram_tensor("b2p", [128, MT], F32, kind="ExternalInput")
    # pout[p, mt*B+n] = paramsT[mt*128+p, n]
    pout = nc.dram_tensor("pout", [128, MT * B], F32, kind="ExternalOutput")

    NW2C = 3  # W2 column chunks (DMA/compute overlap)
    CW = SH // NW2C  # 1536 columns per chunk
    MT_PER_C = CW // 128  # 12

    with tile.TileContext(nc) as tc:
        with (
            tc.tile_pool(name="const", bufs=1) as const,
            tc.tile_pool(name="psum", bufs=1, space="PSUM") as psum,
        ):
            # PE warm-up during the W2 DMA so param matmuls run warm.
            junk = const.tile([128, 128], BF16, tag="junk")
            nc.gpsimd.memset(junk[:], 0.0)
            jps = psum.tile([128, 128], F32, tag="jps")
            for i in range(36):
                nc.tensor.matmul(jps[:], junk[:], junk[:],
                                 start=(i == 0), stop=(i == 35),
                                 skip_group_check=True)

            # Small inputs first so they don't queue behind the W2 slabs.
            xT_sb = const.tile([128, KO, B], F32, tag="xT")
            nc.sync.dma_start(xT_sb.rearrange("p ko n -> p (ko n)"), xp.ap())
            W1_sb = const.tile([128, KO, HID], F32, tag="W1")
            nc.sync.dma_start(W1_sb.rearrange("p ko m -> p (ko m)"), w1p.ap())
            b1a = const.tile([128, 1], F32, tag="b1a")
            nc.sync.dma_start(b1a[:], b1.ap()[0:128, :])
            b1b = const.tile([64, 1], F32, tag="b1b")
            nc.sync.dma_start(b1b[:], b1.ap()[128:HID, :])
            b2_sb = const.tile([128, MT], F32, tag="b2")
            nc.sync.dma_start(b2_sb[:], b2p.ap())

            # W2 slice (bf16), chunked; rows 0-127 and 128-191 separately.
            w2a = []
            w2b = []
            for c in range(NW2C):
                ta = const.tile([128, CW], BF16, tag=f"w2a{c}")
                nc.sync.dma_start(ta[:], W2s.ap()[0:128, c * CW:(c + 1) * CW])
                w2a.append(ta)
                tb = const.tile([64, CW], BF16, tag=f"w2b{c}")
                nc.sync.dma_start(tb[:], W2s.ap()[128:HID, c * CW:(c + 1) * CW])
                w2b.append(tb)

            # hT = relu(W1.T @ x.T + b1), shape (192, 16) as 128 + 64 rows.
            ph1 = psum.tile([128, B], F32, tag="ph")
            for k in range(KO):
                nc.tensor.matmul(ph1[:], W1_sb[:, k, 0:128], xT_sb[:, k, :],
                                 start=(k == 0), stop=(k == KO - 1))
            ph2 = psum.tile([64, B], F32, tag="ph2")
            for k in range(KO):
                nc.tensor.matmul(ph2[:], W1_sb[:, k, 128:HID], xT_sb[:, k, :],
                                 start=(k == 0), stop=(k == KO - 1))
            hT1 = const.tile([128, B], F32, tag="hT1")
            nc.scalar.activation(hT1[:], ph1[:], AF.Relu, bias=b1a[:])
            hT2 = const.tile([64, B], F32, tag="hT2")
            nc.scalar.activation(hT2[:], ph2[:], AF.Relu, bias=b1b[:])
            hb1 = const.tile([128, B], BF16, tag="hb1")
            nc.vector.tensor_copy(out=hb1[:], in_=hT1[:])
            hb2 = const.tile([64, B], BF16, tag="hb2")
            nc.vector.tensor_copy(out=hb2[:], in_=hT2[:])

            # paramsT tile mt = tanh(W2s[:, mt].T @ hT + b2s[mt])
            outp = const.tile([128, MT, B], F32, tag="outp")
            for mt in range(MT):
                c, i = divmod(mt, MT_PER_C)
                pp = psum.tile([128, B], F32, tag="pp", bufs=5)
                nc.tensor.matmul(pp[:], w2a[c][:, i * 128:(i + 1) * 128],
                                 hb1[:], start=True, stop=False)
                nc.tensor.matmul(pp[:], w2b[c][:, i * 128:(i + 1) * 128],
                                 hb2[:], start=False, stop=True)
                nc.scalar.activation(outp[:, mt, :], pp[:], AF.Tanh,
                                     bias=b2_sb[:, mt:mt + 1])
            nc.sync.dma_start(pout.ap(), outp.rearrange("p mt n -> p (mt n)"))

    nc.compile()
    return nc


def build_phase_b():
    nc = bacc.Bacc("TRN2", target_bir_lowering=False, debug=False,
                   num_devices=NCORES)
    feat = nc.dram_tensor("feat", [2, CIN, H, W], F32, kind="ExternalInput")
    # Pair weights wp[p, s, kx, co]: for sample A (s=0) partitions are
    # (ky=0 ci | ky=1 ci); for sample B (s=1) they are (ky=1 | ky=0) --
    # matching the flipped plane layout below. ws[p, kx, co] holds the
    # ky=2 taps: partitions (A ci | B ci).
    wp = nc.dram_tensor("wp", [128, 2, K, COUT], BF16, kind="ExternalInput")
    ws = nc.dram_tensor("ws", [128, K, COUT], BF16, kind="ExternalInput")
    out = nc.dram_tensor("out", [2, COUT, H, W], F32, kind="ExternalOutput")
    featp = feat.ap().rearrange("s c r x -> (s c) r x")  # [128, 112, 112]
    outp = out.ap().rearrange("s c r x -> (s c) r x")

    # Output-row bands: a small first band fills the pipeline quickly.
    BANDS = [(0, 8), (8, 24), (32, 28), (60, 28), (88, 24)]
    NBD = len(BANDS)

    with tile.TileContext(nc) as tc:
        with (
            tc.tile_pool(name="const", bufs=1) as const,
            tc.tile_pool(name="bands", bufs=1) as bands,
            tc.tile_pool(name="outs", bufs=2) as outs,
            tc.tile_pool(name="psum", bufs=1, space="PSUM") as psum,
        ):
            wpair = const.tile([128, 2, K, COUT], BF16, tag="wpair")
            nc.sync.dma_start(wpair[:], wp.ap())
            wsing = const.tile([128, K, COUT], BF16, tag="wsing")
            nc.sync.dma_start(wsing[:], ws.ap())

            # PE warm-up: ~3.5us of junk matmuls so HAM is at full clock
            # when the first real matmul issues.
            junk = const.tile([128, 128], BF16, tag="junk")
            nc.gpsimd.memset(junk[:], 0.0)
            jps = psum.tile([128, 128], F32, tag="jps")
            for i in range(36):
                nc.tensor.matmul(jps[:], junk[:], junk[:],
                                 start=(i == 0), stop=(i == 35),
                                 skip_group_check=True)

            # Raw fp32 feature bands (residual source + cast source), one
            # 128-partition DMA per band on the SP ring (FIFO: band 0
            # lands first). Plane band b covers padded rows [s0, s0+n+3)
            # = feature rows [s0-1, s0+n+2) clipped; local lp = padded-s0.
            # planeA: partitions 0-63 = F (sample A), 64-127 = G = F one
            # row up. planeB flipped: 0-63 = G (sample B), 64-127 = F.
            f32b, plA, plB = [], [], []
            for b, (s0, n) in enumerate(BANDS):
                r0 = max(0, s0 - 1)
                r1 = min(H, s0 + n + 2)
                t32 = bands.tile([128, r1 - r0, W], F32, tag=f"f32b{b}",
                                 name=f"f32b{b}")
                nc.sync.dma_start(t32[:], featp[:, r0:r1, :])
                f32b.append(t32)
                plA.append(bands.tile([128, n + 3, HP], BF16, tag=f"plA{b}",
                                      name=f"plA{b}"))
                plB.append(bands.tile([128, n + 3, HP], BF16, tag=f"plB{b}",
                                      name=f"plB{b}"))

            for b, (s0, n) in enumerate(BANDS):
                r0 = max(0, s0 - 1)
                r1 = min(H, s0 + n + 2)
                PR = n + 3
                lp0 = 1 if b == 0 else 0
                nr = r1 - r0
                fa = plA[b][0:64]     # F plane, sample A
                fb = plB[b][64:128]   # F plane, sample B
                # zero borders of the F planes on the otherwise-idle
                # GPSIMD (G inherits zeros via the shift copy)
                for f in (fa, fb):
                    nc.gpsimd.memset(f[:, :, 0:1], 0.0)
                    nc.gpsimd.memset(f[:, :, HP - 1:HP], 0.0)
                    if b == 0:
                        nc.gpsimd.memset(f[:, 0:1, :], 0.0)
                    if b == NBD - 1:
                        nc.gpsimd.memset(f[:, lp0 + nr:PR, :], 0.0)
                # f32 -> bf16 casts: sample A on DVE, sample B on ACT
                nc.vector.tensor_copy(out=fa[:, lp0:lp0 + nr, 1:1 + W],
                                      in_=f32b[b][0:64])
                nc.scalar.mul(fb[:, lp0:lp0 + nr, 1:1 + W],
                              f32b[b][64:128], 1.0)
                # G = F shifted one row up (SBUF->SBUF DMA, ACT ring)
                nc.scalar.dma_start(plA[b][64:128, 0:PR - 1, :],
                                    plA[b][0:64, 1:PR, :])
                nc.scalar.dma_start(plB[b][0:64, 0:PR - 1, :],
                                    plB[b][64:128, 1:PR, :])

            for b, (s0, n) in enumerate(BANDS):
                r0 = max(0, s0 - 1)
                cpb = n // CH
                ob = outs.tile([128, n, W], F32, tag=f"ob{b % 2}",
                               name=f"ob{b}")
                pss = [psum.tile([128, CH, W], F32, tag="ps", bufs=7,
                                 name=f"ps{b}_{j}") for j in range(cpb)]
                for t in range(2 * K):  # 3 pair slots then 3 single slots
                    kx = t % K
                    for j in range(cpb):
                        for s in range(2):
                            sl = slice(s * 64, (s + 1) * 64)
                            pl = (plA, plB)[s][b]
                            if t < K:  # ky={0,1} pair, K=128
                                lhsT = wpair[:, s, kx, :]
                                rhs = pl[:, CH * j:CH * j + CH, kx:kx + W]
                            else:  # ky=2 single, K=64 on the F plane
                                lhsT = wsing[sl, kx, :]
                                rhs = pl[sl, CH * j + 2:CH * j + 2 + CH,
                                         kx:kx + W]
                            nc.tensor.matmul(
                                pss[j][sl], lhsT, rhs,
                                start=(t == 0), stop=(t == 2 * K - 1),
                                tile_position=(0 if t < K else s * 64,
                                               s * 64),
                                skip_group_check=True)
                for j in range(cpb):
                    y0 = s0 + CH * j  # global output row
                    nc.vector.tensor_add(
                        out=ob[:, CH * j:CH * j + CH, :], in0=pss[j][:],
                        in1=f32b[b][:, y0 - r0:y0 - r0 + CH, :])
                    if b == NBD - 1 and j % 2 == 1:
                        # stream the last band out in quarters to cut the
                        # kernel tail
                        nc.scalar.dma_start(
                            outp[:, y0 - CH:y0 + CH, :],
                            ob[:, CH * (j - 1):CH * (j + 1), :])
                if b != NBD - 1:
                    nc.scalar.dma_start(outp[:, s0:s0 + n, :], ob[:])

    nc.compile()
    return nc


def prep_a_inputs(cls_token, W1, b1, W2, b2):
    x = cls_token[:, 0, :]  # (16, 768)
    xp = np.ascontiguousarray(
        x.T.reshape(KO, 128, B).transpose(1, 0, 2).reshape(128, KO * B))
    w1p = np.ascontiguousarray(
        W1.reshape(KO, 128, HID).transpose(1, 0, 2).reshape(128, KO * HID))
    b1c = np.ascontiguousarray(b1.reshape(HID, 1))
    W2b = W2.astype(ml_dtypes.bfloat16)
    in_a = []
    for j in range(NCORES):
        sl = slice(j * SH, (j + 1) * SH)
        in_a.append({
            "xp": xp,
            "w1p": w1p,
            "b1": b1c,
            "W2s": np.ascontiguousarray(W2b[:, sl]),
            "b2p": np.ascontiguousarray(b2[sl].reshape(MT, 128).T),
        })
    return in_a


def params_from_a(res_a):
    # pout[p, mt*B+n] = paramsT[mt*128+p, n] -> (TOTAL, B)
    slabs = []
    for j in range(NCORES):
        po = res_a.results[j]["pout"].reshape(128, MT, B)
        slabs.append(po.transpose(1, 0, 2).reshape(SH, B))
    return np.concatenate(slabs, axis=0)


def wT_from_params(paramsT):
    # rows are (co, ci, ky, kx). Build per-core pair/single weight slabs:
    #   T[s, ky, ci, kx, co] = w[s][co, ci, ky, kx]
    T = np.ascontiguousarray(
        paramsT.reshape(COUT, CIN, K, K, B).transpose(4, 2, 1, 3, 0)
    ).astype(ml_dtypes.bfloat16)
    wps, wss = [], []
    for j in range(NCORES):
        A, Bm = T[2 * j], T[2 * j + 1]
        wpc = np.empty((128, 2, K, COUT), dtype=ml_dtypes.bfloat16)
        wpc[:64, 0] = A[0]; wpc[64:, 0] = A[1]   # A: (F=ky0 | G=ky1)
        wpc[:64, 1] = Bm[1]; wpc[64:, 1] = Bm[0]  # B flipped: (G=ky1 | F=ky0)
        wsc = np.empty((128, K, COUT), dtype=ml_dtypes.bfloat16)
        wsc[:64] = A[2]; wsc[64:] = Bm[2]
        wps.append(np.ascontiguousarray(wpc))
        wss.append(np.ascontiguousarray(wsc))
    return wps, wss


def prep_b_inputs(features, wT):
    wps, wss = wT
    return [
        {"feat": features[2 * j:2 * j + 2], "wp": wps[j], "ws": wss[j]}
        for j in range(NCORES)
    ]


_cache = {}


def _get(name, builder):
    if name not in _cache:
        _cache[name] = builder()
    return _cache[name]


def kernel(cls_token, features, W1, b1, W2, b2):
    cls_token = np.asarray(cls_token, dtype=np.float32)
    features = np.ascontiguousarray(np.asarray(features, dtype=np.float32))
    W1 = np.ascontiguousarray(np.asarray(W1, dtype=np.float32))
    b1 = np.asarray(b1, dtype=np.float32)
    W2 = np.asarray(W2, dtype=np.float32)
    b2 = np.asarray(b2, dtype=np.float32)

    ncA = _get("A", build_phase_a)
    ncB = _get("B", build_phase_b)
    cores = list(range(NCORES))

    in_a = prep_a_inputs(cls_token, W1, b1, W2, b2)
    res_a = run_bass_kernel_spmd(ncA, in_a, core_ids=cores)
    paramsT = params_from_a(res_a)
    wT = wT_from_params(paramsT)

    in_b = prep_b_inputs(features, wT)
    res_b = run_bass_kernel_spmd(ncB, in_b, core_ids=cores)
    out = np.concatenate(
        [res_b.results[j]["out"] for j in range(NCORES)], axis=0)
    return out
